# revision 16
# baseline (speedup 1.0000x reference)
"""Trainium2 Bass kernel for KL-divergence 1-NN label lookup (AnchorStore).

reference:
    self[k]  = mean_d a[k,d]*log a[k,d]
    cross    = einsum('kd,bd->kb', a, log q) / D
    kl[b,k]  = self[k] - cross[k,b]
    out[b]   = queue_label[argmin_k kl[b,k]]

Strategy (8 NeuronCores, D-sharded, fp16 operands):
    Each core owns a D-slice (padded with 1.0 so log()=0 contributes
    nothing), shipped as fp16 in d-tile-major layout [128, NT, K].
    Working in SUM units (scale-invariant for argmin):
        m[b,k] = sum_d lq[d,b]*at[d,k] - sum_d at[d,k]*log(at[d,k])
    K is split into P passes of KW columns so each pass's
    ReduceScatter(add) overlaps the next pass's compute.  Per pass:
      - TensorE: stationary lq tiles [128d,128b] x moving at [128d,KW]
        accumulate cross into PSUM; the -self term accumulates via a
        (-1)-stationary x pair-summed t = at*log(at) (DVE adds d-tile
        pairs in fp16 to halve the self-matmul column count).
      - ScalarE computes log() (Ln activation) in large batches.
      - Drain: m = pk + srep -> DRAM -> ReduceScatter -> per-pass
        argmax partials (value + label via the is_equal trick).
    Final: combine the P per-pass (value,label) columns, emit 32 int32
    labels per core; host concatenates.
"""

import os
import sys

import numpy as np

sys.path.insert(0, "/opt/trn_rl_repo")

from concourse import bacc, bass, mybir, tile  # noqa: E402
from concourse import bass_utils  # noqa: E402

K = 2048
B = 256
D = 50257
NCORES = 8
NT = 50             # d-tiles of 128 per core (padded)
DSH = NT * 128      # 6400
BS = B // NCORES    # 32 queries per core after ReduceScatter
F32 = mybir.dt.float32
F16 = mybir.dt.float16


def build(mm_dtype=F16, passes=4, bt=8, pair=True, warm_cc=True):
    """Build the SPMD Bass graph for one core (all cores identical)."""
    P = passes
    KW = K // P              # k columns per pass
    ncl = KW // 512 if KW >= 512 else 0   # full-512 chunks per pass
    assert KW % 512 == 0 or KW in (256,), KW
    nc = bacc.Bacc(
        "TRN2", target_bir_lowering=False, debug=False, num_devices=NCORES
    )
    at_d = nc.dram_tensor("at", [128, NT, K], mm_dtype, kind="ExternalInput")
    qt_d = nc.dram_tensor("qt", [128, NT, B], mm_dtype, kind="ExternalInput")
    lab_d = nc.dram_tensor("lab1", [BS, K], F32, kind="ExternalInput")
    out_d = nc.dram_tensor("out", [BS], mybir.dt.int32, kind="ExternalOutput")

    LN = mybir.ActivationFunctionType.Ln
    AX = mybir.AxisListType.X
    OP = mybir.AluOpType

    # d-tile batches (per pass): groups of `bt` tiles, even-sized for
    # pairs; the first batch is small so the first matmuls start early
    batches = [(0, 2)]
    t0 = 2
    while t0 < NT:
        t1 = min(t0 + bt, NT)
        batches.append((t0, t1))
        t0 = t1

    # q chunks for lq computation (front chunks small for fast start)
    qch = [(0, 6), (6, 14), (14, 30), (30, NT)]

    with tile.TileContext(nc) as tc:
        with (
            tc.tile_pool(name="const", bufs=1) as constp,
            tc.tile_pool(name="lqp", bufs=1) as lqp,
            tc.tile_pool(name="qinp", bufs=2) as qinp,
            tc.tile_pool(name="atp", bufs=4) as atp,
            tc.tile_pool(name="latp", bufs=3) as latp,
            tc.tile_pool(name="ttp", bufs=3) as ttp,
            tc.tile_pool(name="tpp", bufs=2) as tpp,
            tc.tile_pool(name="msbp", bufs=2) as msbp,
            tc.tile_pool(name="epp", bufs=2) as epp,
            tc.tile_pool(name="psp", bufs=1, space="PSUM") as psp,
            tc.tile_pool(name="dramp", bufs=1, space="DRAM") as dramp,
        ):
            # --- constants / warmup ---------------------------------
            # Dummy Ln on a tiny memset buffer: forces the ACT table
            # load to happen at t=0, hidden under the first DMAs.
            dum = constp.tile([128, 16], F32)
            nc.gpsimd.memset(dum[:], 1.0)
            dumo = constp.tile([128, 16], F32)
            nc.scalar.activation(dumo[:], dum[:], LN)

            negones_f = constp.tile([128, 128], F32)
            nc.gpsimd.memset(negones_f[:], -1.0)
            negones = constp.tile([128, 128], mm_dtype)
            nc.vector.tensor_copy(negones[:], negones_f[:])

            if warm_cc:
                # Tiny dummy collective issued up front with no input
                # dependencies: pre-warms ncfw/credit state on the CC
                # engine and doubles as the cross-core rendezvous long
                # before the first real ReduceScatter.
                w_in = dramp.tile([1, 64], F32)
                w_out = dramp.tile([NCORES, 64], F32)
                w_sb = constp.tile([1, 64], F32)
                nc.gpsimd.memset(w_sb[:], 1.0)
                nc.gpsimd.dma_start(w_in[:], w_sb[:])
                nc.gpsimd.collective_compute(
                    "AllGather",
                    OP.bypass,
                    replica_groups=[list(range(NCORES))],
                    ins=[w_in.opt()],
                    outs=[w_out.opt()],
                )

            lab1 = constp.tile([BS, K], F32)
            nc.gpsimd.dma_start(lab1[:], lab_d[:])

            # --- lq = log(query^T), fp16, resident -------------------
            lq = lqp.tile([128, NT, B], mm_dtype)
            qsb = []
            for ci, (c0_, c1_) in enumerate(qch):
                qtile = qinp.tile(
                    [128, c1_ - c0_, B], mm_dtype, name=f"qtile_{ci}",
                    tag="qtile",
                )
                qsb.append((qtile, c0_, c1_))
            # chunk 0 DMA + ACT first so the first matmuls start early;
            # qt rides the scalar engine's DMA queue so it never queues
            # behind the att stream on sync.
            nc.scalar.dma_start(qsb[0][0][:], qt_d[:, qch[0][0]:qch[0][1], :])
            nc.scalar.activation(
                lq[:, qch[0][0]:qch[0][1], :], qsb[0][0][:], LN
            )

            # --- PSUM accumulators (parity double-buffered) ----------
            pk = {}
            srep = {}
            for par in range(min(2, P)):
                for bti in range(2):
                    for cl in range(max(1, ncl)):
                        pk[(par, bti, cl)] = psp.tile(
                            [128, min(KW, 512)], F32,
                            name=f"pk_{par}_{bti}_{cl}",
                            tag=f"pk_{par}_{bti}_{cl}",
                        )
                for cl in range(max(1, ncl)):
                    srep[(par, cl)] = psp.tile(
                        [128, min(KW, 512)], F32, name=f"srep_{par}_{cl}",
                        tag=f"srep_{par}_{cl}",
                    )

            # per-pass (value, label) partials, combined at the end
            vcat = epp.tile([BS, P], F32, bufs=1)
            lcat = epp.tile([BS, P], F32, bufs=1)

            qt_dma_emitted = 1  # chunk 0 already emitted

            # Two collective groups: passes [0, P-1) share one big
            # ReduceScatter (launched after pass P-2, fully overlapped
            # by pass P-1 compute); the last pass gets a small RS on
            # the critical tail.  This keeps the CC engine far from
            # saturation and minimises tail latency.
            KWA = (P - 1) * KW
            ar_a = dramp.tile([B, KWA], F32, name="ar_a", bufs=1)
            rs_a = dramp.tile([BS, KWA], F32, name="rs_a", bufs=1)
            ar_b = dramp.tile([B, KW], F32, name="ar_b", bufs=1)
            rs_b = dramp.tile([BS, KW], F32, name="rs_b", bufs=1)

            for ps in range(P):
                par = ps % 2
                k0 = ps * KW
                nclp = max(1, ncl)
                cw = min(KW, 512)
                for bi, (tb0, tb1) in enumerate(batches):
                    n = tb1 - tb0
                    att = atp.tile(
                        [128, n, KW], mm_dtype, name=f"att_{ps}_{bi}",
                        tag="att",
                    )
                    nc.sync.dma_start(
                        att[:], at_d[:, tb0:tb1, k0:k0 + KW]
                    )
                    latt = latp.tile(
                        [128, n, KW], F32 if False else mm_dtype,
                        name=f"latt_{ps}_{bi}", tag="latt",
                    )
                    nc.scalar.activation(latt[:], att[:], LN)
                    # trickle in remaining q chunks between early batches
                    if ps == 0 and qt_dma_emitted < len(qch):
                        qtile, c0_, c1_ = qsb[qt_dma_emitted]
                        nc.scalar.dma_start(qtile[:], qt_d[:, c0_:c1_, :])
                        nc.scalar.activation(
                            lq[:, c0_:c1_, :], qtile[:], LN
                        )
                        qt_dma_emitted += 1
                    tt = ttp.tile(
                        [128, n, KW], mm_dtype, name=f"tt_{ps}_{bi}",
                        tag="tt",
                    )
                    nc.vector.tensor_tensor(tt[:], att[:], latt[:],
                                            op=OP.mult)
                    # cross matmuls
                    for j in range(n):
                        t = tb0 + j
                        for bti in range(2):
                            lhs = lq[:, t, bti * 128:(bti + 1) * 128]
                            for cl in range(nclp):
                                nc.tensor.matmul(
                                    pk[(par, bti, cl)][:],
                                    lhs,
                                    att[:, j, cl * cw:(cl + 1) * cw],
                                    start=(t == 0),
                                    stop=(t == NT - 1),
                                )
                    # self term: pair-sum tt across d-tiles, then matmul
                    if pair:
                        npair = n // 2
                        tp = tpp.tile(
                            [128, npair, KW], mm_dtype,
                            name=f"tp_{ps}_{bi}", tag="tp",
                        )
                        for i in range(npair):
                            nc.vector.tensor_tensor(
                                tp[:, i, :], tt[:, 2 * i, :],
                                tt[:, 2 * i + 1, :], op=OP.add,
                            )
                        first = bi == 0
                        last = bi == len(batches) - 1
                        for i in range(npair):
                            for cl in range(nclp):
                                nc.tensor.matmul(
                                    srep[(par, cl)][:],
                                    negones[:],
                                    tp[:, i, cl * cw:(cl + 1) * cw],
                                    start=(first and i == 0),
                                    stop=(last and i == npair - 1),
                                )
                    else:
                        first = bi == 0
                        last = bi == len(batches) - 1
                        for j in range(n):
                            for cl in range(nclp):
                                nc.tensor.matmul(
                                    srep[(par, cl)][:],
                                    negones[:],
                                    tt[:, j, cl * cw:(cl + 1) * cw],
                                    start=(first and j == 0),
                                    stop=(last and j == n - 1),
                                )

                # --- drain pass ps: PSUM -> SBUF -> DRAM -------------
                last_grp = ps == P - 1
                ar_in = ar_b if last_grp else ar_a
                koff = 0 if last_grp else ps * KW
                for cl in range(nclp):
                    srep_sb = msbp.tile(
                        [128, cw], F32, name=f"srep_sb_{ps}_{cl}",
                        tag="srep_sb",
                    )
                    nc.vector.tensor_copy(srep_sb[:], srep[(par, cl)][:])
                    for bti in range(2):
                        m_sb = msbp.tile(
                            [128, cw], F32, name=f"m_sb_{ps}_{cl}_{bti}",
                            tag=f"m_sb{bti}",
                        )
                        nc.vector.tensor_tensor(
                            m_sb[:], pk[(par, bti, cl)][:], srep_sb[:],
                            op=OP.add,
                        )
                        nc.gpsimd.dma_start(
                            ar_in[
                                bti * 128:(bti + 1) * 128,
                                koff + cl * cw:koff + (cl + 1) * cw,
                            ],
                            m_sb[:],
                        )
                if ps == P - 2:
                    nc.gpsimd.collective_compute(
                        "ReduceScatter",
                        OP.add,
                        replica_groups=[list(range(NCORES))],
                        ins=[ar_a.opt()],
                        outs=[rs_a.opt()],
                    )
                elif last_grp:
                    nc.gpsimd.collective_compute(
                        "ReduceScatter",
                        OP.add,
                        replica_groups=[list(range(NCORES))],
                        ins=[ar_b.opt()],
                        outs=[rs_b.opt()],
                    )
            # --- tail: msum loads + per-slice epilogues -------------
            msum_a = epp.tile([BS, KWA], F32, bufs=1)
            nc.gpsimd.dma_start(msum_a[:], rs_a[:])
            msum_b = epp.tile([BS, KW], F32, bufs=1)
            nc.gpsimd.dma_start(msum_b[:], rs_b[:])

            def emit_epi(ps_, msum, moff):
                k0_ = ps_ * KW
                nc.vector.tensor_reduce(
                    vcat[:, ps_:ps_ + 1], msum[:, moff:moff + KW],
                    axis=AX, op=OP.max,
                )
                eqt = epp.tile(
                    [BS, KW], F32, name=f"eq_{ps_}", tag="eq",
                )
                nc.vector.tensor_scalar(
                    eqt[:], msum[:, moff:moff + KW], vcat[:, ps_:ps_ + 1],
                    None, op0=OP.is_equal,
                )
                cand = epp.tile(
                    [BS, KW], F32, name=f"cand_{ps_}", tag="cand",
                )
                nc.vector.tensor_tensor(
                    cand[:], eqt[:], lab1[:, k0_:k0_ + KW], op=OP.mult
                )
                nc.vector.tensor_reduce(
                    lcat[:, ps_:ps_ + 1], cand[:], axis=AX, op=OP.max
                )

            for ps_ in range(P - 1):
                emit_epi(ps_, msum_a, ps_ * KW)
            emit_epi(P - 1, msum_b, 0)

            # --- final combine across passes ------------------------
            vg = epp.tile([BS, 1], F32, bufs=1)
            nc.vector.tensor_reduce(vg[:], vcat[:], axis=AX, op=OP.max)
            eqp = epp.tile([BS, P], F32, bufs=1)
            nc.vector.tensor_scalar(
                eqp[:], vcat[:], vg[:], None, op0=OP.is_equal
            )
            candp = epp.tile([BS, P], F32, bufs=1)
            nc.vector.tensor_tensor(candp[:], eqp[:], lcat[:], op=OP.mult)
            lmax = epp.tile([BS, 1], F32, bufs=1)
            nc.vector.tensor_reduce(lmax[:], candp[:], axis=AX, op=OP.max)
            labf = epp.tile([BS, 1], F32, bufs=1)
            nc.vector.tensor_scalar_add(labf[:], lmax[:], -1.0)
            labi = epp.tile([BS, 1], mybir.dt.int32, bufs=1)
            nc.vector.tensor_copy(labi[:], labf[:])
            nc.scalar.dma_start(out_d[:], labi[:])

    nc.compile()
    return nc


def shard_inputs(query, queue_anchor, queue_label, dsh=DSH, d_real=D):
    """Host-side layout prep: pad D with 1.0 (log 1 = 0), per-core
    d-tile-major fp16 slices [128, NT, K]; label row replicated."""
    np_dt = np.float16
    q = np.asarray(query, np.float32)
    a = np.asarray(queue_anchor, np.float32)
    lab1 = (np.asarray(queue_label).astype(np.float32) + 1.0)[None, :]
    lab1 = np.ascontiguousarray(np.broadcast_to(lab1, (BS, lab1.shape[1])))
    in_maps = []
    for c in range(NCORES):
        lo = c * dsh
        hi = min((c + 1) * dsh, d_real)
        at = np.ones((dsh, a.shape[0]), np_dt)
        qt = np.ones((dsh, q.shape[0]), np_dt)
        if hi > lo:
            at[: hi - lo, :] = a[:, lo:hi].T.astype(np_dt)
            qt[: hi - lo, :] = q[:, lo:hi].T.astype(np_dt)
        # [dsh, X] -> tile-major [128, NT, X]
        at = np.ascontiguousarray(
            at.reshape(NT, 128, -1).transpose(1, 0, 2)
        )
        qt = np.ascontiguousarray(
            qt.reshape(NT, 128, -1).transpose(1, 0, 2)
        )
        in_maps.append({"at": at, "qt": qt, "lab1": lab1})
    return in_maps


def unshard_out(per_core_outs, split_rs=False):
    """Reassemble the 8 cores' 32-label slices into the [256] output."""
    return np.concatenate([np.asarray(o) for o in per_core_outs])


_NC_CACHE = {}


def _split_rs_active():
    return False


def _get_nc():
    key = (
        os.environ.get("ANCHOR_MM_DTYPE", "float16"),
        int(os.environ.get("ANCHOR_PASSES", "4")),
        int(os.environ.get("ANCHOR_BT", "8")),
        os.environ.get("ANCHOR_PAIR", "1") == "1",
        os.environ.get("ANCHOR_WARM_CC", "1") == "1",
    )
    if key not in _NC_CACHE:
        _NC_CACHE[key] = build(
            mm_dtype=getattr(mybir.dt, key[0]), passes=key[1], bt=key[2],
            pair=key[3], warm_cc=key[4],
        )
    return _NC_CACHE[key]


def kernel(query, queue_anchor, queue_label):
    nc = _get_nc()
    in_maps = shard_inputs(query, queue_anchor, queue_label)
    res = bass_utils.run_bass_kernel_spmd(
        nc, in_maps, core_ids=list(range(NCORES))
    )
    out = unshard_out([res.results[i]["out"] for i in range(NCORES)])
    return out.astype(np.asarray(queue_label).dtype)
